# revision 34
# baseline (speedup 1.0000x reference)
"""Multi-head attention (B=4, S=2048, D=1024, H=16) on 8 Trainium2 cores.

Sharding: data-parallel over batch (4) x tensor-parallel over heads (2 groups
of 8 heads). Core c handles batch c//2, head-group c%2. Each core computes
column-parallel QKV projections for its 8 heads, attention, and a row-parallel
output projection producing a partial [D, S] (transposed) result. The host
sums the two head-group partials per batch, transposes, and adds bo.

On-device layout notes:
- Activations are kept feature-major (X^T, Q^T, K^T, out^T) so every matmul's
  operands are natural [K=128, free] SBUF tiles; the host pre-transposes the
  inputs and post-transposes the output (free on CPU).
- All matmuls run in fp32r (full-rate fp32 mode of the PE).
- scores^T = K_h Q_h^T is computed per head with K=64 contraction; the two
  heads of a 128-partition tile are row-packed via tile_position (0,0)/(64,0)
  into the two halves of one 2-bank PSUM tile, so a single scalar-engine Exp
  (with the 1/sqrt(dk) scale fused) covers the pair.
- softmax runs in the [k, q] layout: the denominators come for free from the
  PV matmul through a ones column appended to V (M=65); normalization is
  applied to the PV output (linearity) via fast reciprocal + gpsimd
  partition-broadcast + vector multiply.
- Emission order interleaves Q-projection bands and the (deferred) output
  projection with the attention loop so the scalar engine starts exp work
  early and the PE never drains at block boundaries.
"""

import sys

sys.path.insert(0, "/opt/trn_rl_repo")
import numpy as np

S = 2048
D = 1024
DH = 512  # per-core head dims (8 heads x 64)
DK = 64
NPAIR = 4  # head pairs per core
QB = 512  # q block
NQB = S // QB
NKT = S // 128  # 16 k tiles for attention
SCALE = 1.0 / np.sqrt(DK)

_CACHE = {}


def _build():
    import concourse.mybir as mybir
    import concourse.tile as tile
    from concourse import bacc

    F32 = mybir.dt.float32
    F32R = mybir.dt.float32r
    AF = mybir.ActivationFunctionType
    MULT = mybir.AluOpType.mult

    nc = bacc.Bacc(None, target_bir_lowering=False, debug=False)

    xqT = nc.dram_tensor("xqT", [D, S], F32R, kind="ExternalInput")
    xkT = nc.dram_tensor("xkT", [D, S], F32R, kind="ExternalInput")
    xvT = nc.dram_tensor("xvT", [D, S], F32R, kind="ExternalInput")
    wq = nc.dram_tensor("wq", [D, DH], F32R, kind="ExternalInput")
    wk = nc.dram_tensor("wk", [D, DH], F32R, kind="ExternalInput")
    wv = nc.dram_tensor("wv", [D, DH], F32R, kind="ExternalInput")
    wo = nc.dram_tensor("wo", [DH, D], F32R, kind="ExternalInput")
    bq = nc.dram_tensor("bq", [DH], F32, kind="ExternalInput")
    bk = nc.dram_tensor("bk", [DH], F32, kind="ExternalInput")
    bv = nc.dram_tensor("bv", [DH], F32, kind="ExternalInput")
    outT = nc.dram_tensor("outT", [D, S], F32, kind="ExternalOutput")

    with tile.TileContext(nc) as tc:
        with (
            tc.tile_pool(name="res", bufs=1) as res,  # resident tensors
            tc.tile_pool(name="stream", bufs=1) as stream,
            tc.tile_pool(name="work", bufs=1) as work,
            tc.tile_pool(name="ps", bufs=1, space="PSUM") as ps,
        ):
            # ---- resident tiles ----
            qt = [
                [
                    res.tile([128, QB], F32R, tag=f"qt{p}_{n}", name=f"qt{p}_{n}")
                    for n in range(NQB)
                ]
                for p in range(NPAIR)
            ]
            kt = [res.tile([128, S], F32R, tag=f"kt{p}", name=f"kt{p}") for p in range(NPAIR)]
            # V tiles: [128 seq, 8*65] -- per local head 64 features + ones col
            vt = [res.tile([128, 8 * 65], F32R, tag=f"vt{m}", name=f"vt{m}") for m in range(NKT)]
            wot = [res.tile([128, D], F32R, tag=f"wo{p}", name=f"wo{p}") for p in range(NPAIR)]

            onesf = res.tile([128, 8], F32, tag="onesf")
            nc.vector.memset(onesf[:], 1.0)
            # biases as [128, 1] per feature tile
            bqt = res.tile([128, NPAIR], F32, tag="bqt")
            bkt = res.tile([128, NPAIR], F32, tag="bkt")
            bvt = res.tile([128, NPAIR], F32, tag="bvt")
            for p in range(NPAIR):
                nc.sync.dma_start(out=bqt[:, p : p + 1], in_=bq[p * 128 : (p + 1) * 128])
                nc.sync.dma_start(out=bkt[:, p : p + 1], in_=bk[p * 128 : (p + 1) * 128])
                nc.sync.dma_start(out=bvt[:, p : p + 1], in_=bv[p * 128 : (p + 1) * 128])

            def load_w(w_d, label):
                tiles = []
                for k in range(8):
                    wt_ = stream.tile([128, DH], F32R, tag="ws", bufs=8, name=f"w_{label}{k}")
                    eng = nc.sync if k % 2 == 0 else nc.gpsimd
                    eng.dma_start(out=wt_[:], in_=w_d[k * 128 : (k + 1) * 128, :])
                    tiles.append(wt_)
                return tiles

            def load_x_block(x_d, n, label):
                # 8 chunks [128, 512] covering columns n*512 : (n+1)*512
                chunks = []
                for k in range(8):
                    xt_ = stream.tile([128, 512], F32R, tag="xs", bufs=12, name=f"x_{label}{n}_{k}")
                    eng = nc.sync if k % 2 == 0 else nc.gpsimd
                    eng.dma_start(
                        out=xt_[:],
                        in_=x_d[k * 128 : (k + 1) * 128, n * 512 : (n + 1) * 512],
                    )
                    chunks.append(xt_)
                return chunks

            # ---- V projection: sequence-major [S, 512] + interleaved ones ----
            vw = load_w(wv, "v")
            for mg in range(4):  # groups of 4 m-tiles per [128,512] chunk set
                vx = load_x_block(xvT, mg, "v")
                for mi in range(4):
                    m = mg * 4 + mi
                    psum = ps.tile([128, 512], F32, tag="pv", bufs=2, name=f"ps_v{m}")
                    for k in range(8):
                        nc.tensor.matmul(
                            psum[:],
                            lhsT=vx[k][:, mi * 128 : (mi + 1) * 128],
                            rhs=vw[k][:],
                            start=(k == 0),
                            stop=(k == 7),
                        )
                    vview = vt[m][:].rearrange("p (h d) -> p h d", d=65)
                    with nc.allow_low_precision(reason="f32r rounding for PE"):
                        # bv is NOT added here; it is folded in post-softmax
                        nc.vector.tensor_copy(
                            vview[:, :, 0:64],
                            psum[:].rearrange("p (h d) -> p h d", d=64),
                        )
                        nc.vector.tensor_copy(
                            vview[:, :, 64:65],
                            onesf[:].rearrange("p (h d) -> p h d", d=1),
                        )

            # ---- feature-major projection block (one 512-col band) ----
            def proj_block(x_d, wtiles, bias_t, out_aps, n, label, ptag="pv"):
                # out_aps: list over ft of the destination AP for this band
                chunks = load_x_block(x_d, n, label)
                for ft in range(NPAIR):
                    psum = ps.tile(
                        [128, 512], F32, tag=ptag, bufs=2, name=f"ps_{label}{ft}_{n}"
                    )
                    for k in range(8):
                        nc.tensor.matmul(
                            psum[:],
                            lhsT=wtiles[k][:, ft * 128 : (ft + 1) * 128],
                            rhs=chunks[k][:],
                            start=(k == 0),
                            stop=(k == 7),
                        )
                    with nc.allow_low_precision(reason="f32r rounding for PE"):
                        nc.vector.tensor_scalar_add(
                            out_aps[ft],
                            psum[:],
                            bias_t[:, ft : ft + 1],
                        )

            # K projection (all bands), then Q band 0
            wkt = load_w(wk, "k")
            for kb in range(NQB):
                proj_block(
                    xkT, wkt, bkt,
                    [kt[ft][:, kb * 512 : (kb + 1) * 512] for ft in range(NPAIR)],
                    kb, "k",
                )
            wqt = load_w(wq, "q")
            proj_block(xqT, wqt, bqt, [qt[ft][0][:] for ft in range(NPAIR)], 0, "q")

            # wo resident tiles (needed from the first output projection)
            for p in range(NPAIR):
                eng = nc.sync if p % 2 == 0 else nc.gpsimd
                eng.dma_start(out=wot[p][:], in_=wo[p * 128 : (p + 1) * 128, :])

            # ---- attention + interleaved Q band 1 and output projections ----
            pending_proj = []

            def emit_out_proj(n, at_tiles):
                qsl = slice(n * 512, (n + 1) * 512)
                for m in range(8):
                    psum = ps.tile([128, 512], F32, tag="sc", bufs=2, name=f"po{n}_{m}")
                    for p in range(NPAIR):
                        nc.tensor.matmul(
                            psum[:],
                            lhsT=wot[p][:, m * 128 : (m + 1) * 128],
                            rhs=at_tiles[p][:],
                            start=(p == 0),
                            stop=(p == NPAIR - 1),
                        )
                    ot = work.tile([128, 512], F32, tag="ot", bufs=3, name=f"ot{n}_{m}")
                    nc.vector.tensor_copy(ot[:], psum[:])
                    nc.sync.dma_start(out=outT[m * 128 : (m + 1) * 128, qsl], in_=ot[:])

            for n in range(NQB):
                qsl = slice(n * 512, (n + 1) * 512)
                at = []
                for p in range(NPAIR):
                    at.append(work.tile([128, 512], F32R, tag="at", bufs=8, name=f"at{n}_{p}"))
                    h0, h1 = 2 * p, 2 * p + 1
                    # pv pair tile: head0 in cols 0:512, head1 in 512:1024
                    pvt_ = ps.tile([128, 1024], F32, tag="pv", bufs=2, name=f"pv_{n}_{p}")
                    for k in range(NKT):
                        ksl = slice(k * 128, (k + 1) * 128)
                        sc = ps.tile([128, 1024], F32, tag="sc", bufs=2, name=f"sc_{n}_{p}_{k}")
                        nc.tensor.matmul(
                            sc[:, 0:512], lhsT=kt[p][0:64, ksl], rhs=qt[p][n][0:64, :],
                            start=True, stop=True, tile_position=(0, 0),
                        )
                        nc.tensor.matmul(
                            sc[:, 512:1024], lhsT=kt[p][64:128, ksl], rhs=qt[p][n][64:128, :],
                            start=True, stop=True, tile_position=(64, 0),
                        )
                        ex = work.tile([128, 1024], F32R, tag="ex", bufs=3, name=f"ex_{n}_{p}_{k}")
                        with nc.allow_low_precision(reason="f32r exp for PE"):
                            nc.scalar.activation(ex[:], sc[:], AF.Exp, scale=float(SCALE))
                        nc.tensor.matmul(
                            pvt_[0:65, 0:512],
                            lhsT=vt[k][:, h0 * 65 : (h0 + 1) * 65],
                            rhs=ex[:, 0:512],
                            start=(k == 0), stop=(k == NKT - 1),
                        )
                        nc.tensor.matmul(
                            pvt_[0:65, 512:1024],
                            lhsT=vt[k][:, h1 * 65 : (h1 + 1) * 65],
                            rhs=ex[:, 512:1024],
                            start=(k == 0), stop=(k == NKT - 1),
                        )
                    # normalization: at[h] = pv[h][0:64] * (1/denom) + bv
                    rc = work.tile([1, 1024], F32, tag="rc", bufs=2, name=f"rc{n}_{p}")
                    nc.vector.reciprocal(rc[0:1, 0:512], pvt_[64:65, 0:512])
                    nc.vector.reciprocal(rc[0:1, 512:1024], pvt_[64:65, 512:1024])
                    bc0 = work.tile([64, 512], F32, tag="bc", bufs=2, name=f"bc0_{n}_{p}")
                    bc1 = work.tile([64, 512], F32, tag="bc", bufs=2, name=f"bc1_{n}_{p}")
                    nc.gpsimd.partition_broadcast(bc0[:], rc[0:1, 0:512])
                    nc.gpsimd.partition_broadcast(bc1[:], rc[0:1, 512:1024])
                    tmp = work.tile([128, 512], F32, tag="tmp", bufs=2, name=f"tmp{n}_{p}")
                    nc.vector.tensor_tensor(tmp[0:64, :], pvt_[0:64, 0:512], bc0[:], MULT)
                    nc.vector.tensor_tensor(tmp[64:128, :], pvt_[0:64, 512:1024], bc1[:], MULT)
                    with nc.allow_low_precision(reason="f32r rounding for PE"):
                        nc.vector.tensor_scalar_add(at[p][:], tmp[:], bvt[:, p : p + 1])
                    if p == 0 and pending_proj:
                        with tc.high_priority(offset=-150):
                            emit_out_proj(*pending_proj.pop())
                    if p == 1 and n + 1 < NQB:
                        with tc.high_priority(offset=-150):
                            proj_block(
                                xqT, wqt, bqt,
                                [qt[ft][n + 1][:] for ft in range(NPAIR)],
                                n + 1, "q", ptag="sc",
                            )
                pending_proj.append((n, at))
            emit_out_proj(*pending_proj.pop())

    nc.compile()
    return nc


def _get_nc():
    if "nc" not in _CACHE:
        _CACHE["nc"] = _build()
    return _CACHE["nc"]


def kernel(query, key, value, Wq, bq, Wk, bk, Wv, bv, Wo, bo):
    from concourse.bass_utils import run_bass_kernel_spmd

    nc = _get_nc()
    query = np.asarray(query, dtype=np.float32)
    key = np.asarray(key, dtype=np.float32)
    value = np.asarray(value, dtype=np.float32)
    Wq, Wk, Wv, Wo = (np.asarray(a, dtype=np.float32) for a in (Wq, Wk, Wv, Wo))
    bq, bk, bv, bo = (np.asarray(a, dtype=np.float32) for a in (bq, bk, bv, bo))
    B = query.shape[0]

    in_maps = []
    for c in range(8):
        b, g = divmod(c, 2)
        sl = slice(g * DH, (g + 1) * DH)
        in_maps.append(
            {
                "xqT": np.ascontiguousarray(query[b].T),
                "xkT": np.ascontiguousarray(key[b].T),
                "xvT": np.ascontiguousarray(value[b].T),
                "wq": np.ascontiguousarray(Wq[:, sl]),
                "wk": np.ascontiguousarray(Wk[:, sl]),
                "wv": np.ascontiguousarray(Wv[:, sl]),
                "wo": np.ascontiguousarray(Wo[sl, :]),
                "bq": np.ascontiguousarray(bq[sl]),
                "bk": np.ascontiguousarray(bk[sl]),
                "bv": np.ascontiguousarray(bv[sl]),
            }
        )
    _CACHE["last_in_maps"] = in_maps
    r = run_bass_kernel_spmd(nc, in_maps, list(range(8)))
    _CACHE["last_results"] = r
    out = np.empty((B, S, D), dtype=np.float32)
    for b in range(B):
        acc = r.results[2 * b]["outT"] + r.results[2 * b + 1]["outT"]
        out[b] = acc.T + bo
    return out
